# revision 9
# baseline (speedup 1.0000x reference)
"""Trainium2 Bass kernel for nn_MHA_2688649527670.

Reference computes, per batch b and head h:
    Q = x Wq_h^T, K = x Wk_h^T, V = x Wv_h^T          ([S, D] each)
    Z = softmax_over_d( (Q K^T / sqrt(D)) V )

There is NO softmax between Q K^T and V, so the chain is associative:
    (Q K^T) V = x * (Wq_h^T Wk_h G Wv_h^T) / sqrt(D),   G = x^T x   ([D, D])

This collapses the O(S^2 D) attention into a [D,D] weight-chain plus one
[S,D]x[D,D] matmul per head — ~15x fewer FLOPs — followed by softmax over
the model dim (free axis).

Sharding: data parallel over batch (4) x tensor parallel over head-groups
(2 groups of 4 heads) = 8 cores. Each core is fully independent (no
collectives): it receives x[b] and the 4-head weight slices, and produces
out[4 heads, S, D].

All matmuls run in fp32 (4 cycles/row on PE). bf16 anywhere in the chain
was measured at 0.7%-6% output error (softmax of ~N(0,45) logits amplifies
matmul error near max-ties), while fp32 end-to-end is ~1e-5.
"""

import numpy as np

import concourse.bass as bass
import concourse.bacc as bacc
import concourse.mybir as mybir
import concourse.tile as tile
from concourse.bass_utils import run_bass_kernel_spmd
from concourse.masks import make_identity

B, S, D, H = 4, 2048, 128, 8
P = 128
HPC = H // 2          # heads per core (tensor parallel over 2 head groups)
NCH = S // P          # 16 s-chunks of 128 rows
N_CORES = 8
SCALE = 1.0 / float(np.sqrt(D))
F32 = mybir.dt.float32

_PROG = None  # cached compiled Bass program (same SPMD program for all cores)


def _build_program():
    nc = bacc.Bacc("TRN2", target_bir_lowering=False, debug=False,
                   num_devices=N_CORES)

    x_d = nc.dram_tensor("x", [S, D], F32, kind="ExternalInput")
    wq_d = nc.dram_tensor("wq", [HPC * D, D], F32, kind="ExternalInput")
    wk_d = nc.dram_tensor("wk", [HPC * D, D], F32, kind="ExternalInput")
    wv_d = nc.dram_tensor("wv", [HPC * D, D], F32, kind="ExternalInput")
    out_d = nc.dram_tensor("out", [HPC, S, D], F32, kind="ExternalOutput")

    with tile.TileContext(nc) as tc:
        with (
            tc.tile_pool(name="const", bufs=1) as const,
            tc.tile_pool(name="chain", bufs=2) as chain,
            tc.tile_pool(name="work", bufs=6) as work,
            tc.tile_pool(name="ps_y", bufs=3, space="PSUM") as ps_y,
            tc.tile_pool(name="ps_t", bufs=2, space="PSUM") as ps_t,
            tc.tile_pool(name="ps_g", bufs=1, space="PSUM") as ps_g,
            tc.tile_pool(name="ps_c", bufs=2, space="PSUM") as ps_c,
        ):
            ident = const.tile([P, P], F32, tag="ident")
            make_identity(nc, ident)

            # ---- loads ----
            x_sb = const.tile([P, NCH, D], F32, tag="x_sb")
            x_view = x_d.ap().rearrange("(n p) c -> p n c", p=P)
            # split into 4 DMAs so transposes/G can start early
            for q in range(4):
                eng = nc.sync if q % 2 == 0 else nc.scalar
                eng.dma_start(x_sb[:, q * 4:(q + 1) * 4, :],
                              x_view[:, q * 4:(q + 1) * 4, :])

            w_sb = {}
            for nm, wd in (("wq", wq_d), ("wk", wk_d), ("wv", wv_d)):
                t = const.tile([P, HPC, D], F32, tag=f"{nm}_sb", name=f"{nm}_sb")
                nc.sync.dma_start(t, wd.ap().rearrange("(h p) c -> p h c", p=P))
                w_sb[nm] = t

            # ---- G = x^T x (accumulated over 16 s-chunks) ----
            # emitted first so the chain (and then the finals) start ASAP;
            # transposes fill PE gaps while G waits on x-chunk DMAs
            g_ps = ps_g.tile([P, P], F32, tag="g_ps")
            for i in range(NCH):
                nc.tensor.matmul(g_ps, lhsT=x_sb[:, i, :], rhs=x_sb[:, i, :],
                                 start=(i == 0), stop=(i == NCH - 1))
            g_sb = const.tile([P, P], F32, tag="g_sb")
            nc.vector.tensor_copy(g_sb, g_ps)

            # ---- per-head chain: M_h = Wq^T Wk G Wv^T / sqrt(D) ----
            # P0T[a,c] = sum_e Wk[e,a] Wq[e,c]
            # UT[b,c]  = sum_a G[a,b] P0T[a,c]
            # M[c,d]   = sum_b UT[b,c] WvT[b,d]
            m_all = const.tile([P, HPC, D], F32, tag="m_all")
            for h in range(HPC):
                p0t_ps = ps_c.tile([P, P], F32, tag="c_ps")
                nc.tensor.matmul(p0t_ps, lhsT=w_sb["wk"][:, h, :],
                                 rhs=w_sb["wq"][:, h, :])
                p0t_sb = chain.tile([P, P], F32, tag="p0t_sb")
                nc.vector.tensor_copy(p0t_sb, p0t_ps)

                ut_ps = ps_c.tile([P, P], F32, tag="c_ps")
                nc.tensor.matmul(ut_ps, lhsT=g_sb, rhs=p0t_sb)
                ut_sb = chain.tile([P, P], F32, tag="ut_sb")
                nc.vector.tensor_copy(ut_sb, ut_ps)

                wvt_ps = ps_c.tile([P, P], F32, tag="c_ps")
                nc.tensor.transpose(wvt_ps, w_sb["wv"][:, h, :], ident)
                wvt_sb = chain.tile([P, P], F32, tag="wvt_sb")
                nc.vector.tensor_copy(wvt_sb, wvt_ps)

                m_ps = ps_c.tile([P, P], F32, tag="c_ps")
                nc.tensor.matmul(m_ps, lhsT=ut_sb, rhs=wvt_sb)
                nc.scalar.mul(m_all[:, h, :], m_ps, SCALE)

            # ---- xT (PE transpose, 128x128 chunks) ----
            xT_sb = const.tile([P, NCH, D], F32, tag="xT_sb")
            for i in range(NCH):
                tp = ps_t.tile([P, P], F32, tag="tp")
                nc.tensor.transpose(tp, x_sb[:, i, :], ident)
                nc.vector.tensor_copy(xT_sb[:, i, :], tp)

            # ---- final: Y = x @ M (all 4 heads in one N=512 matmul),
            #      then softmax over d per head ----
            m_flat = m_all[:].rearrange("p h d -> p (h d)")
            for i in range(NCH):
                y_ps = ps_y.tile([P, HPC * D], F32, tag="y_ps")
                nc.tensor.matmul(y_ps, lhsT=xT_sb[:, i, :], rhs=m_flat)

                negmax = work.tile([P, HPC], F32, tag="negmax")
                nc.vector.reduce_max(
                    out=negmax,
                    in_=y_ps[:].rearrange("p (h d) -> p h d", h=HPC),
                    axis=mybir.AxisListType.X, negate=True)

                e_sb = work.tile([P, HPC, D], F32, tag="e_sb")
                sums = work.tile([P, HPC], F32, tag="sums")
                for h in range(HPC):
                    nc.scalar.activation(
                        e_sb[:, h, :], y_ps[:, h * D:(h + 1) * D],
                        mybir.ActivationFunctionType.Exp,
                        bias=negmax[:, h:h + 1], scale=1.0,
                        accum_out=sums[:, 0:1] if h == 0 else None)
                nc.vector.reduce_sum(out=sums[:, 1:HPC], in_=e_sb[:, 1:HPC, :],
                                     axis=mybir.AxisListType.X)

                rsum = work.tile([P, HPC], F32, tag="rsum")
                nc.vector.reciprocal(rsum, sums)

                o_sb = work.tile([P, HPC, D], F32, tag="o_sb")
                nc.gpsimd.tensor_tensor(
                    o_sb, e_sb, rsum[:, :, None].to_broadcast((P, HPC, D)),
                    mybir.AluOpType.mult)
                # one DMA per chunk: DRAM walked (s, h, c) to match SBUF (p, h, c)
                nc.sync.dma_start(
                    out_d.ap()[:, i * P:(i + 1) * P, :].rearrange("h s c -> s h c"),
                    o_sb)

    nc.compile()
    return nc


def _get_program():
    global _PROG
    if _PROG is None:
        _PROG = _build_program()
    return _PROG


def _make_in_maps(x, W_q, W_k, W_v):
    in_maps = []
    for core in range(N_CORES):
        b, hg = core // 2, core % 2
        sl = slice(hg * HPC * D, (hg + 1) * HPC * D)
        in_maps.append({
            "x": np.ascontiguousarray(x[b]),
            "wq": np.ascontiguousarray(W_q[sl]),
            "wk": np.ascontiguousarray(W_k[sl]),
            "wv": np.ascontiguousarray(W_v[sl]),
        })
    return in_maps


def run(x, W_q, W_k, W_v, trace=False, **spmd_kwargs):
    """Run on 8 NeuronCores; returns (Z, BassKernelResults)."""
    nc = _get_program()
    in_maps = _make_in_maps(np.asarray(x, np.float32), np.asarray(W_q, np.float32),
                            np.asarray(W_k, np.float32), np.asarray(W_v, np.float32))
    res = run_bass_kernel_spmd(nc, in_maps, core_ids=list(range(N_CORES)),
                               trace=trace, **spmd_kwargs)
    Z = np.empty((B, H, S, D), np.float32)
    for core in range(N_CORES):
        b, hg = core // 2, core % 2
        Z[b, hg * HPC:(hg + 1) * HPC] = np.asarray(res.results[core]["out"])
    return Z, res


def kernel(x, W_q, W_k, W_v):
    Z, _ = run(x, W_q, W_k, W_v, trace=False)
    return Z
